# revision 1
# baseline (speedup 1.0000x reference)
"""RBF Gram matrix kernel for TRN2: out[i,j] = exp(-||x_i - y_j||^2).

x, y: [8192, 64] fp32 -> out [8192, 8192] fp32.

Sharding: x rows split across 8 NeuronCores (1024 rows each), y replicated.
Each core computes a [1024, 8192] tile of the Gram matrix.

Math: s = |x|^2 + |y|^2 - 2 x.y is accumulated in PSUM by two bf16
matmuls using a hi/lo mantissa split (x = xh + xl, y = yh + yl):
  MM1 (K=128): [xh; xl]^T   @ [2yh; 2yh]          -> 2(xh+xl).yh
  MM2 (K=68):  [xh; 1; 1; xsq_h; xsq_l]^T
               @ [2yl; -ysq_h; -ysq_l; -1; -1]    -> 2 xh.yl - |y|^2 - |x|^2
(The dropped xl.yl term is ~1e-4 relative.) PSUM then holds -s, and one
ScalarE Exp pass writes exp(-s) to SBUF as bf16, which DMAs to HBM.

The output leaves the device as bf16 (the host widens to fp32): every
value is exp(-s) with s in ~[29, 250] for this input distribution, so
bf16's 8-bit mantissa costs at most 2^-9 relative to the output absmax --
well inside the 2e-2 budget -- while halving the dominant HBM write.
With the store at 16 MiB/core the kernel is bound by the ScalarE Exp
pass (~1 elem/lane/cycle @ 1.2 GHz over 8.4M elems/core ~= 59 us).
"""

import numpy as np
import ml_dtypes

import concourse.bass as bass
import concourse.tile as tile
import concourse.mybir as mybir
from concourse.bass_utils import run_bass_kernel_spmd

N_CORES = 8
N_ROWS = 8192          # x rows (Gram rows), sharded
N_COLS = 8192          # y rows (Gram cols), replicated
D = 64
RPC = N_ROWS // N_CORES  # 1024 rows per core

DT = mybir.dt.float32
BF = mybir.dt.bfloat16
K1 = 2 * D             # 128: [xh; xl] rows
K2 = D + 4             # 68: [xh; 1; 1; xsq_h; xsq_l] rows
W = RPC + N_COLS       # packed input width: lhsT cols then rhs cols

R_TILES = RPC // 128   # 8 row tiles of 128 partitions
CBW = 2048             # column block width (4 PSUM banks)
MM_W = 512             # one matmul free dim (1 PSUM bank, fp32)
C_BLOCKS = N_COLS // CBW


def _split_excess_waits(nc, limits=None):
    """The walrus in this container accepts only a small number of sync-wait
    commands per instruction (1 for Drain, ~2 elsewhere). Hoist excess waits
    onto injected NoOps on the same engine, placed just before the original
    instruction so per-engine ordering (and thus the waits) is preserved."""
    if limits is None:
        limits = {"InstNoOp": 1, "default": 1}
    n_split = 0
    for f in nc.m.functions:
        for blk in f.blocks:
            insts = blk.instructions
            i = 0
            while i < len(insts):
                inst = insts[i]
                si = inst.sync_info
                lim = limits.get(type(inst).__name__, limits["default"])
                if si is not None and len(si.on_wait) > lim:
                    waits = list(si.on_wait)
                    keep = waits[-lim:] if lim > 0 else []
                    excess = waits[:-lim] if lim > 0 else waits
                    per_nop = limits["InstNoOp"]
                    chunks = [
                        excess[j:j + per_nop] for j in range(0, len(excess), per_nop)
                    ]
                    for k, ch in enumerate(chunks):
                        nop = mybir.InstNoOp(
                            name=nc.get_next_instruction_name(),
                            sync_info=mybir.SyncInfo(on_wait=ch, on_update=[]),
                            bass_nofuse=True,
                            engine=inst.engine,
                        )
                        nc.register_instruction(nop)
                        insts.insert(i + k, nop)
                    si.on_wait = keep
                    i += len(chunks)
                    n_split += 1
                i += 1
    return n_split


def _dedup_ldweights(nc):
    """Walrus runs with --enable-ldw-opt=false, so every InstMatmult gets
    its own InstLdweights even when consecutive matmuls share the same
    stationary operand -- each reload costs ~70-110 ns of serial PE time.
    Delete an InstLdweights whose weight AP is identical to the previous
    one on the PE stream (nothing else mutates the PE array), merging its
    waits into the next PE instruction. LDWs carry no semaphore updates,
    and waits are >=-monotonic, so the merge preserves synchronization."""
    n = 0
    for f in nc.m.functions:
        for blk in f.blocks:
            insts = blk.instructions
            last_sig = None
            carry = []
            i = 0
            while i < len(insts):
                inst = insts[i]
                t = type(inst).__name__
                if str(inst.engine) != "EngineType.PE":
                    i += 1
                    continue
                if carry:
                    si = inst.sync_info
                    if si is None:
                        inst.sync_info = mybir.SyncInfo(
                            on_wait=carry, on_update=[])
                    else:
                        si.on_wait = list(si.on_wait) + carry
                    carry = []
                if t == "InstLdweights":
                    sig = str(inst.ins[0])
                    si = inst.sync_info
                    assert si is None or not si.on_update
                    if sig == last_sig:
                        carry = list(si.on_wait) if si else []
                        del insts[i]
                        n += 1
                        continue
                    last_sig = sig
                elif t == "InstMatmult":
                    pass
                else:
                    last_sig = None  # unknown PE inst: be conservative
                i += 1
            assert not carry
    return n


def emit_body(nc, sbin, sbout, ps, p1_d, p2_d, out_d,
              mm=True, act=True, store=True):
    """One full pass: input DMAs, matmuls, Exp, output DMAs.
    The mm/act/store flags carve out stages for bottleneck ablation."""
    NCH = N_COLS // CBW  # rhs DMA chunk width == column block width

    # chunked input tiles: the first matmuls gate on the lhs chunks
    # plus one 2048-col rhs chunk instead of the whole 3.45 MiB input
    p1_lhs = sbin.tile([K1, RPC], BF, name="p1l")
    p2_lhs = sbin.tile([K2, RPC], BF, name="p2l")
    p1_rhs = [sbin.tile([K1, CBW], BF, name=f"p1r{h}") for h in range(NCH)]
    p2_rhs = [sbin.tile([K2, CBW], BF, name=f"p2r{h}") for h in range(NCH)]
    # input DMAs ride the two HWDGE rings (SP and ACT) rather than SWDGE:
    # same SDMA engines, and it keeps GpSimd entirely out of the kernel
    # body. Outputs must also split across both rings -- one ring alone
    # cannot sustain the 16 MiB store (measured 98.3 us vs 90.4 us).
    nc.sync.dma_start(p1_lhs[:], p1_d[:, :RPC])
    nc.scalar.dma_start(p2_lhs[:], p2_d[:, :RPC])
    for h in range(NCH):
        c0 = RPC + h * CBW
        nc.sync.dma_start(p1_rhs[h][:], p1_d[:, c0:c0 + CBW])
        nc.scalar.dma_start(p2_rhs[h][:], p2_d[:, c0:c0 + CBW])

    n_dma = 0
    for r in range(R_TILES):
        lhs1 = p1_lhs[:, r * 128:(r + 1) * 128]
        lhs2 = p2_lhs[:, r * 128:(r + 1) * 128]
        for cb in range(C_BLOCKS):
            acc = ps.tile([128, CBW], DT)
            if mm:
                # group by stationary operand: 4x lhs1 then 4x lhs2, so
                # the PE reloads weights twice per block instead of 8x
                for j in range(CBW // MM_W):
                    seg = slice(j * MM_W, (j + 1) * MM_W)
                    nc.tensor.matmul(
                        acc[:, seg], lhs1, p1_rhs[cb][:, seg],
                        start=True, stop=False,
                    )
                for j in range(CBW // MM_W):
                    seg = slice(j * MM_W, (j + 1) * MM_W)
                    nc.tensor.matmul(
                        acc[:, seg], lhs2, p2_rhs[cb][:, seg],
                        start=False, stop=True,
                    )
            if act:
                ot = sbout.tile([128, CBW], BF)
                nc.scalar.activation(
                    ot[:], acc[:], mybir.ActivationFunctionType.Exp
                )
                if store:
                    # alternate between the two HWDGE rings (SP and ACT)
                    eng = nc.scalar if n_dma % 2 else nc.sync
                    eng.dma_start(
                        out_d[r * 128:(r + 1) * 128,
                              cb * CBW:(cb + 1) * CBW],
                        ot[:],
                    )
            n_dma += 1


def build_nc():
    nc = bass.Bass()
    p1_d = nc.dram_tensor("p1", [K1, W], BF, kind="ExternalInput")
    p2_d = nc.dram_tensor("p2", [K2, W], BF, kind="ExternalInput")
    out_d = nc.dram_tensor("out", [RPC, N_COLS], BF, kind="ExternalOutput")

    with tile.TileContext(nc) as tc:
        with (
            tc.tile_pool(name="inp", bufs=1) as sbin,
            tc.tile_pool(name="outp", bufs=8) as sbout,
            tc.tile_pool(name="ps", bufs=2, space="PSUM") as ps,
        ):
            # warm the ACT exp table-set load (~2.7 us) under the input DMAs
            warm = sbout.tile([128, 8], DT, name="actwarm")
            nc.scalar.activation(warm[:], warm[:], mybir.ActivationFunctionType.Exp)
            emit_body(nc, sbin, sbout, ps, p1_d, p2_d, out_d)
    finalize_nc(nc)
    return nc


def finalize_nc(nc):
    _dedup_ldweights(nc)
    _split_excess_waits(nc)


def _bf(a):
    return a.astype(ml_dtypes.bfloat16)


def prepare_inputs(x, y):
    """Host-side prep: hi/lo split, transpose, pack per-core input maps."""
    x = np.asarray(x, dtype=np.float32)
    y = np.asarray(y, dtype=np.float32)
    assert x.shape == (N_ROWS, D) and y.shape == (N_COLS, D)

    x_sq = (x * x).sum(axis=1, dtype=np.float32)
    y_sq = (y * y).sum(axis=1, dtype=np.float32)

    xh = _bf(x)
    xl = _bf(x - xh.astype(np.float32))
    yh = _bf(y)
    yl2 = _bf(2.0 * (y - yh.astype(np.float32)))
    xsq_h = _bf(x_sq)
    xsq_l = _bf(x_sq - xsq_h.astype(np.float32))
    ysq_h = _bf(y_sq)
    ysq_l = _bf(y_sq - ysq_h.astype(np.float32))

    # rhs halves are shared by all cores
    rhs1 = np.concatenate([2 * yh.T, 2 * yh.T], axis=0).astype(ml_dtypes.bfloat16)
    ones_n = np.ones((1, N_COLS), ml_dtypes.bfloat16)
    rhs2 = np.concatenate(
        [yl2.T, -ysq_h[None, :], -ysq_l[None, :], -ones_n, -ones_n], axis=0
    ).astype(ml_dtypes.bfloat16)

    in_maps = []
    for c in range(N_CORES):
        rows = slice(c * RPC, (c + 1) * RPC)
        ones_m = np.ones((1, RPC), ml_dtypes.bfloat16)
        lhs1 = np.concatenate([xh.T[:, rows], xl.T[:, rows]], axis=0)
        lhs2 = np.concatenate(
            [xh.T[:, rows], ones_m, ones_m,
             xsq_h[None, rows], xsq_l[None, rows]], axis=0
        )
        p1 = np.concatenate([lhs1, rhs1], axis=1).astype(ml_dtypes.bfloat16)
        p2 = np.concatenate([lhs2, rhs2], axis=1).astype(ml_dtypes.bfloat16)
        in_maps.append({"p1": p1, "p2": p2})
    return in_maps


def kernel(x, y):
    in_maps = prepare_inputs(x, y)
    nc = build_nc()
    res = run_bass_kernel_spmd(nc, in_maps, core_ids=list(range(N_CORES)))
    out = np.concatenate([res.results[c]["out"] for c in range(N_CORES)], axis=0)
    return out.astype(np.float32)

